# revision 1
# baseline (speedup 1.0000x reference)
"""DeformableConv2D (DCNv2) forward on 8 Trainium2 NeuronCores.

Data-parallel over batch: one sample per core. Per core: offset conv on the
tensor engine (fp16 operands, fp32 accumulate); sampling coordinates and
bilinear weights on the vector engine; modulated bilinear sampling via SWDGE
dma_gather of 2x2-patch rows; corner combination via broadcast multiply
(split vector/gpsimd) + accumulating PE transposes; im2col GEMM on the
tensor engine.
"""
import sys
sys.path.insert(0, "/opt/trn_rl_repo")

import numpy as np
import ml_dtypes

import concourse.bass as bass
import concourse.bacc as bacc
import concourse.mybir as mybir
import concourse.tile as tile
from concourse import library_config

F32 = mybir.dt.float32
F16 = mybir.dt.float16
I16 = mybir.dt.int16
AL = mybir.AluOpType

H = W = 64
C = 128
F = 256
K = 9
PADR = 8                 # padded-coordinate margin
HP = WP = 80             # padded image
NPIX = H * W             # 4096
NBLK = 32                # pixel blocks of 128 (2 rows each)
CONVW = 66               # conv grid width (pad 1)
CONVN = 4608             # padded conv output length (9 tiles of 512)
XCLM = 67 + CONVN + 67   # xcl with shift margins
NROWS = 2 * HP * 40      # pair-table rows = 6400
NSLOT = 18               # gathered rows per pixel = (k, yc)
NCHUNK = 72              # gather instructions (8 slots x 128 px each)

DY = np.repeat(np.arange(3) - 1, 3).astype(np.float32)   # per-tap dy
DX = np.tile(np.arange(3) - 1, 3).astype(np.float32)     # per-tap dx


def bcast(ap, shape):
    return ap.to_broadcast(list(shape))


_NC = None


def build_nc():
    nc = bacc.Bacc("TRN2", target_bir_lowering=False)
    xcl = nc.dram_tensor("xcl", [C, XCLM], F16, kind="ExternalInput")
    pairs = nc.dram_tensor("pairs", [NROWS, 512], F16, kind="ExternalInput")
    offk = nc.dram_tensor("offk", [C, K * 27], F16, kind="ExternalInput")
    offb = nc.dram_tensor("offb", [27, 1], F32, kind="ExternalInput")
    filt = nc.dram_tensor("filt", [C, K * 2 * 128], F16, kind="ExternalInput")
    eye32 = nc.dram_tensor("eye32", [128, 128], F32, kind="ExternalInput")
    eye16 = nc.dram_tensor("eye16", [128, 128], F16, kind="ExternalInput")
    # consts: Y_all [128,32], dy/dx rows [128,9] each, X_all [128,1]
    consts = nc.dram_tensor("consts", [128, 51], F32, kind="ExternalInput")
    out_d = nc.dram_tensor("out", [2, 128, NPIX], F32, kind="ExternalOutput")

    with tile.TileContext(nc) as tc:
        with (
            tc.tile_pool(name="const", bufs=1) as cpool,
            tc.tile_pool(name="bwork", bufs=1) as bpool,
            tc.tile_pool(name="dram", bufs=1, space="DRAM") as dpool,
            tc.tile_pool(name="ps2", bufs=2, space="PSUM") as ps2pool,
        ):
            nc.gpsimd.load_library(library_config.mlp)

            s_offb = cpool.tile([27, 1], F32)
            nc.sync.dma_start(out=s_offb[:], in_=offb[:])
            s_filt = cpool.tile([C, K * 2 * 128], F16)
            nc.sync.dma_start(out=s_filt[:], in_=filt[:])
            s_eye32 = cpool.tile([128, 128], F32)
            nc.sync.dma_start(out=s_eye32[:], in_=eye32[:])
            s_eye16 = cpool.tile([128, 128], F16)
            nc.sync.dma_start(out=s_eye16[:], in_=eye16[:])
            s_const = cpool.tile([128, 51], F32)
            nc.sync.dma_start(out=s_const[:], in_=consts[:])
            y_all = s_const[:, 0:32]          # [128, 32]
            dy_t = s_const[:, 32:41]          # [128, 9]
            dx_t = s_const[:, 41:50]
            x_all = s_const[:, 50:51]         # [128, 1]

            # survivors of the conv/stage-B phase
            w16 = cpool.tile([128, 32, 36], F16)
            idxw = cpool.tile([128, 2304], I16)     # wrapped gather indices

            with (
                tc.tile_pool(name="conv", bufs=1) as vpool,
                tc.tile_pool(name="ps", bufs=2, space="PSUM") as pspool,
            ):
                # ---- loads ----
                s_xcl = vpool.tile([C, XCLM], F16)
                nc.sync.dma_start(out=s_xcl[:], in_=xcl[:])
                s_offk = vpool.tile([C, K * 27], F16)
                nc.sync.dma_start(out=s_offk[:], in_=offk[:])

                # ---- P1: offset conv (row-pair tiles) + P2 transposes ----
                wi_c = vpool.tile([27, 32, 128], F32)
                wiT = bpool.tile([128, 32, 27], F32)
                for b in range(NBLK):
                    q0 = (2 * b + 1) * CONVW
                    ps = pspool.tile([27, 132], F32, tag="convps")
                    for t in range(K):
                        d = int(DY[t]) * CONVW + int(DX[t])
                        nc.tensor.matmul(
                            out=ps[:],
                            lhsT=s_offk[:, t * 27:(t + 1) * 27],
                            rhs=s_xcl[:, 67 + q0 + d: 67 + q0 + 132 + d],
                            start=(t == 0), stop=(t == K - 1),
                        )
                    nc.vector.tensor_scalar(
                        out=wi_c[:, b, :].rearrange("p (r x) -> p r x", x=64),
                        in0=ps[:].rearrange("p (r x) -> p r x", x=CONVW)[:, :, 1:65],
                        scalar1=s_offb[:, 0:1], scalar2=None, op0=AL.add)
                    pt = pspool.tile([128, 27], F32, tag="wiTps")
                    nc.tensor.transpose(
                        out=pt[:], in_=wi_c[:, b, :], identity=s_eye32[:27, :27])
                    nc.vector.tensor_copy(out=wiT[:, b, :], in_=pt[:])

            # ---- P3: stage B ----
            o1 = wiT[:, :, 0:9]
            o2 = wiT[:, :, 9:18]
            mm = wiT[:, :, 18:27]
            S = [128, 32, 9]

            sigm = bpool.tile(S, F32)
            nc.scalar.activation(sigm[:], mm, mybir.ActivationFunctionType.Sigmoid)

            py = bpool.tile(S, F32, tag="py")
            nc.vector.tensor_tensor(
                out=py[:], in0=o1, in1=bcast(y_all, S),
                op=AL.add)
            nc.vector.tensor_tensor(
                out=py[:], in0=py[:], in1=bcast(dy_t.rearrange("p (o k) -> p o k", o=1), S),
                op=AL.add)
            nc.vector.tensor_scalar(out=py[:], in0=py[:], scalar1=8.0, scalar2=2.0,
                                    op0=AL.add, op1=AL.max)
            nc.vector.tensor_scalar(out=py[:], in0=py[:], scalar1=77.0, scalar2=None, op0=AL.min)
            y0p = bpool.tile(S, F32, tag="y0p")
            nc.vector.tensor_scalar(out=y0p[:], in0=py[:], scalar1=-0.5,
                                    scalar2=8388608.0, op0=AL.add, op1=AL.add)
            nc.vector.tensor_scalar(out=y0p[:], in0=y0p[:], scalar1=-8388608.0,
                                    scalar2=None, op0=AL.add)
            fy = bpool.tile(S, F32, tag="fy")
            nc.vector.tensor_tensor(out=fy[:], in0=py[:], in1=y0p[:], op=AL.subtract)
            wy0 = bpool.tile(S, F32, tag="wy0")
            nc.vector.tensor_scalar(out=wy0[:], in0=fy[:], scalar1=-1.0, scalar2=1.0,
                                    op0=AL.mult, op1=AL.add)

            px = bpool.tile(S, F32, tag="px")
            nc.vector.tensor_tensor(
                out=px[:], in0=o2,
                in1=bcast(x_all, S), op=AL.add)
            nc.vector.tensor_tensor(
                out=px[:], in0=px[:], in1=bcast(dx_t.rearrange("p (o k) -> p o k", o=1), S),
                op=AL.add)
            nc.vector.tensor_scalar(out=px[:], in0=px[:], scalar1=8.0, scalar2=2.0,
                                    op0=AL.add, op1=AL.max)
            nc.vector.tensor_scalar(out=px[:], in0=px[:], scalar1=77.0, scalar2=None, op0=AL.min)
            x0p = bpool.tile(S, F32, tag="x0p")
            nc.vector.tensor_scalar(out=x0p[:], in0=px[:], scalar1=-0.5,
                                    scalar2=8388608.0, op0=AL.add, op1=AL.add)
            nc.vector.tensor_scalar(out=x0p[:], in0=x0p[:], scalar1=-8388608.0,
                                    scalar2=None, op0=AL.add)
            fx = bpool.tile(S, F32, tag="fx")
            nc.vector.tensor_tensor(out=fx[:], in0=px[:], in1=x0p[:], op=AL.subtract)
            wx0 = bpool.tile(S, F32, tag="wx0")
            nc.vector.tensor_scalar(out=wx0[:], in0=fx[:], scalar1=-1.0, scalar2=1.0,
                                    op0=AL.mult, op1=AL.add)

            qx = bpool.tile(S, F32, tag="qx")
            nc.vector.tensor_scalar(out=qx[:], in0=x0p[:], scalar1=0.5,
                                    scalar2=-0.25, op0=AL.mult, op1=AL.add)
            nc.vector.tensor_scalar(out=qx[:], in0=qx[:], scalar1=8388608.0,
                                    scalar2=-8388608.0, op0=AL.add, op1=AL.add)
            parx = bpool.tile(S, F32, tag="parx")
            nc.vector.scalar_tensor_tensor(
                out=parx[:], in0=qx[:], scalar=-2.0, in1=x0p[:],
                op0=AL.mult, op1=AL.add)
            qy = bpool.tile(S, F32, tag="qy")
            nc.vector.tensor_scalar(out=qy[:], in0=y0p[:], scalar1=0.5,
                                    scalar2=-0.25, op0=AL.mult, op1=AL.add)
            nc.vector.tensor_scalar(out=qy[:], in0=qy[:], scalar1=8388608.0,
                                    scalar2=-8388608.0, op0=AL.add, op1=AL.add)
            pary = bpool.tile(S, F32, tag="pary")
            nc.vector.scalar_tensor_tensor(
                out=pary[:], in0=qy[:], scalar=-2.0, in1=y0p[:],
                op0=AL.mult, op1=AL.add)
            base = bpool.tile(S, F32, tag="base")
            nc.vector.scalar_tensor_tensor(
                out=base[:], in0=qy[:], scalar=40.0, in1=qx[:],
                op0=AL.mult, op1=AL.add)
            nc.vector.scalar_tensor_tensor(
                out=base[:], in0=parx[:], scalar=1600.0, in1=base[:],
                op0=AL.mult, op1=AL.add)
            nc.vector.scalar_tensor_tensor(
                out=base[:], in0=pary[:], scalar=3200.0, in1=base[:],
                op0=AL.mult, op1=AL.add)

            idx_i16 = bpool.tile([128, 32, 9], I16)
            nc.vector.tensor_copy(out=idx_i16[:], in_=base[:])

            # wrapped-index staging: DRAM round trip
            idx_dram = dpool.tile([128, 288], I16)
            nc.sync.dma_start(out=idx_dram[:],
                              in_=idx_i16[:].rearrange("p b k -> p (b k)"))
            # wrapped[q + 16rep, (g, pg)] = flatidx[(pg*16+q)*288 + g]
            for g in range(8):
                nc.sync.dma_start(
                    out=idxw[16 * g:16 * (g + 1), :],
                    in_=idx_dram[:].rearrange(
                        "(pg q) g -> q g pg", pg=8))

            # weights W [128, 32, 9, 2, 2]  (k, yc, u)
            a0 = bpool.tile(S, F32, tag="a0")
            nc.vector.tensor_tensor(out=a0[:], in0=wy0[:], in1=sigm[:], op=AL.mult)
            a1 = bpool.tile(S, F32, tag="a1")
            nc.vector.tensor_tensor(out=a1[:], in0=fy[:], in1=sigm[:], op=AL.mult)
            w_f32 = bpool.tile([128, 32, 9, 2, 2], F32)
            nc.vector.tensor_tensor(out=w_f32[:, :, :, 0, 0], in0=a0[:], in1=wx0[:],
                                    op=AL.mult)
            nc.vector.tensor_tensor(out=w_f32[:, :, :, 0, 1], in0=a0[:], in1=fx[:],
                                    op=AL.mult)
            nc.vector.tensor_tensor(out=w_f32[:, :, :, 1, 0], in0=a1[:], in1=wx0[:],
                                    op=AL.mult)
            nc.vector.tensor_tensor(out=w_f32[:, :, :, 1, 1], in0=a1[:], in1=fx[:],
                                    op=AL.mult)
            nc.vector.tensor_copy(
                out=w16[:], in_=w_f32[:].rearrange("p b k y u -> p b (k y u)"))

            # ---- P4 ----
            with (
                tc.tile_pool(name="sgpool", bufs=2) as sgpool,
                tc.tile_pool(name="blkpool", bufs=2) as blkpool,
                tc.tile_pool(name="ps3", bufs=4, space="PSUM") as ps3pool,
            ):
                for sg in range(8):
                    dst = sgpool.tile([128, 36, 512], F16, tag="dst")
                    g0 = sg * 36
                    for cl in range(5):
                        lo = 8 * cl
                        ns = min(8, 36 - lo)
                        nc.gpsimd.dma_gather(
                            dst[:, lo:lo + ns, :], pairs[:],
                            idxw[:, (g0 + lo) * 8:(g0 + lo + ns) * 8],
                            ns * 128, ns * 128, 512)
                    cols = sgpool.tile([128, K, 512], F16, tag="cols")
                    for bi in range(4):
                        b = sg * 4 + bi
                        gw = blkpool.tile([128, 36, 128], F16, tag="gw")
                        dsrc = dst[:, 9 * bi:9 * (bi + 1), :].rearrange(
                            "p s e -> p (s e)").rearrange(
                            "p (j c) -> p j c", c=128)
                        nc.vector.tensor_tensor(
                            out=gw[:, 0:24, :], in0=dsrc[:, 0:24, :],
                            in1=bcast(w16[:, b, 0:24], [128, 24, 128]),
                            op=AL.mult)
                        nc.gpsimd.tensor_tensor(
                            out=gw[:, 24:36, :], in0=dsrc[:, 24:36, :],
                            in1=bcast(w16[:, b, 24:36], [128, 12, 128]),
                            op=AL.mult)
                        for k in range(K):
                            pc = ps3pool.tile([128, 128], F32, tag="ctps")
                            for j in range(4):
                                nc.tensor.matmul(
                                    out=pc[:], lhsT=gw[:, 4 * k + j, :],
                                    rhs=s_eye16[:], start=(j == 0), stop=(j == 3))
                            if k % 2 == 0:
                                nc.scalar.copy(
                                    out=cols[:, k, bi * 128:(bi + 1) * 128],
                                    in_=pc[:])
                            else:
                                nc.vector.tensor_copy(
                                    out=cols[:, k, bi * 128:(bi + 1) * 128],
                                    in_=pc[:])
                    for fc in range(2):
                        po = ps2pool.tile([128, 512], F32, tag="outps")
                        for k in range(K):
                            nc.tensor.matmul(
                                out=po[:],
                                lhsT=s_filt[:, (k * 2 + fc) * 128:
                                            (k * 2 + fc + 1) * 128],
                                rhs=cols[:, k, :],
                                start=(k == 0), stop=(k == K - 1))
                        osb = blkpool.tile([128, 512], F32, tag="osb")
                        if fc == 0:
                            nc.scalar.copy(out=osb[:], in_=po[:])
                        else:
                            nc.vector.tensor_copy(out=osb[:], in_=po[:])
                        nc.sync.dma_start(
                            out=out_d[fc, :, sg * 512:(sg + 1) * 512], in_=osb[:])
    nc.compile()
    return nc


def host_inputs(x, offset_kernel, offset_bias, filt_w):
    """Per-sample input maps. x [8,64,64,128] f32 etc (numpy)."""
    offk = np.ascontiguousarray(
        offset_kernel.reshape(K, C, 27).transpose(1, 0, 2).reshape(C, K * 27)
    ).astype(np.float16)
    offb = offset_bias.reshape(27, 1).astype(np.float32)
    filt_re = np.ascontiguousarray(
        filt_w.reshape(K, C, 2, 128).transpose(1, 0, 2, 3).reshape(C, K * 2 * 128)
    ).astype(np.float16)
    eye32 = np.eye(128, dtype=np.float32)
    eye16 = np.eye(128).astype(np.float16)
    consts = np.zeros((128, 51), np.float32)
    p = np.arange(128)
    yoff = p // 64
    consts[:, 0:32] = 2 * np.arange(32)[None, :] + yoff[:, None]
    consts[:, 32:41] = DY[None, :]
    consts[:, 41:50] = DX[None, :]
    consts[:, 50] = p % 64

    maps = []
    for b in range(x.shape[0]):
        xp = np.zeros((HP + 2, WP + 2, C), np.float32)
        xp[PADR:PADR + H, PADR:PADR + W] = x[b]
        quad = np.zeros((2, 2, 40, 40, 2, 2, C), np.float32)
        for pY in range(2):
            for pX in range(2):
                for uy in range(2):
                    for ux in range(2):
                        quad[pY, pX, :, :, uy, ux] = \
                            xp[pY + uy:pY + uy + 80:2, pX + ux:pX + ux + 80:2]
        prs = quad.reshape(NROWS, 4 * C).astype(np.float16)

        x1 = np.zeros((CONVW, CONVW, C), np.float32)
        x1[1:65, 1:65] = x[b]
        xcl = np.zeros((C, XCLM), np.float16)
        xcl[:, 67:67 + 4356] = x1.reshape(CONVW * CONVW, C).T.astype(np.float16)
        maps.append({
            "xcl": xcl, "pairs": prs, "offk": offk, "offb": offb,
            "filt": filt_re, "eye32": eye32, "eye16": eye16, "consts": consts,
        })
    return maps


def host_output(res_list):
    outs = []
    for r in res_list:
        o = r["out"].reshape(256, NPIX)
        outs.append(np.ascontiguousarray(o.T).reshape(H, W, F))
    return np.stack(outs)


def _get_nc():
    global _NC
    if _NC is None:
        _NC = build_nc()
    return _NC


def kernel(inputs, offset_kernel, offset_bias, filt):
    from concourse.bass_utils import run_bass_kernel_spmd
    x = np.asarray(inputs, dtype=np.float32)
    maps = host_inputs(x, np.asarray(offset_kernel, np.float32),
                       np.asarray(offset_bias, np.float32),
                       np.asarray(filt, np.float32))
    nc = _get_nc()
    res = run_bass_kernel_spmd(nc, maps, core_ids=list(range(8)))
    return host_output(res.results).astype(np.float32)



# revision 4
# speedup vs baseline: 1.4946x; 1.4946x over previous
"""DeformableConv2D (DCNv2) forward on 8 Trainium2 NeuronCores.

Data-parallel over batch: one sample per core. Per core: offset conv on the
tensor engine (fp16 operands, fp32 accumulate); sampling coordinates and
bilinear weights on the vector engine; modulated bilinear sampling via SWDGE
dma_gather of 2x2-patch rows; corner combination via broadcast multiply
(split vector/gpsimd) + accumulating PE transposes; im2col GEMM on the
tensor engine.
"""
import sys
sys.path.insert(0, "/opt/trn_rl_repo")

import numpy as np
import ml_dtypes

import concourse.bass as bass
import concourse.bacc as bacc
import concourse.mybir as mybir
import concourse.tile as tile
from concourse import library_config

F32 = mybir.dt.float32
F16 = mybir.dt.float16
I16 = mybir.dt.int16
AL = mybir.AluOpType

H = W = 64
C = 128
F = 256
K = 9
PADR = 8                 # padded-coordinate margin
HP = WP = 80             # padded image
NPIX = H * W             # 4096
NBLK = 32                # pixel blocks of 128 (2 rows each)
CONVW = 66               # conv grid width (pad 1)
CONVN = 4608             # padded conv output length (9 tiles of 512)
XCLM = 67 + CONVN + 67   # xcl with shift margins
NROWS = 2 * HP * 40      # pair-table rows = 6400
NSLOT = 18               # gathered rows per pixel = (k, yc)
NCHUNK = 72              # gather instructions (8 slots x 128 px each)

DY = np.repeat(np.arange(3) - 1, 3).astype(np.float32)   # per-tap dy
DX = np.tile(np.arange(3) - 1, 3).astype(np.float32)     # per-tap dx


def bcast(ap, shape):
    return ap.to_broadcast(list(shape))


_NC = None


def build_nc():
    nc = bacc.Bacc("TRN2", target_bir_lowering=False)
    xcl = nc.dram_tensor("xcl", [C, XCLM], F16, kind="ExternalInput")
    pairs = nc.dram_tensor("pairs", [NROWS, 512], F16, kind="ExternalInput")
    offk = nc.dram_tensor("offk", [C, K * 27], F16, kind="ExternalInput")
    offb = nc.dram_tensor("offb", [27, 1], F32, kind="ExternalInput")
    filt = nc.dram_tensor("filt", [C, K * 2 * 128], F16, kind="ExternalInput")
    eye32 = nc.dram_tensor("eye32", [128, 128], F32, kind="ExternalInput")
    eye16 = nc.dram_tensor("eye16", [128, 128], F16, kind="ExternalInput")
    # consts: Y_all [128,32], dy/dx rows [128,9] each, X_all [128,1]
    consts = nc.dram_tensor("consts", [128, 51], F32, kind="ExternalInput")
    out_d = nc.dram_tensor("out", [2, 128, NPIX], F32, kind="ExternalOutput")

    with tile.TileContext(nc) as tc:
        with (
            tc.tile_pool(name="const", bufs=1) as cpool,
            tc.tile_pool(name="bwork", bufs=1) as bpool,
            tc.tile_pool(name="ps2", bufs=2, space="PSUM") as ps2pool,
        ):
            nc.gpsimd.load_library(library_config.mlp)

            s_offb = cpool.tile([27, 1], F32)
            nc.sync.dma_start(out=s_offb[:], in_=offb[:])
            s_filt = cpool.tile([C, K * 2 * 128], F16)
            nc.sync.dma_start(out=s_filt[:], in_=filt[:])
            s_eye32 = cpool.tile([128, 128], F32)
            nc.sync.dma_start(out=s_eye32[:], in_=eye32[:])
            s_eye16 = cpool.tile([128, 128], F16)
            nc.sync.dma_start(out=s_eye16[:], in_=eye16[:])
            s_const = cpool.tile([128, 51], F32)
            nc.sync.dma_start(out=s_const[:], in_=consts[:])
            y_all = s_const[:, 0:32]          # [128, 32]
            dy_t = s_const[:, 32:41]          # [128, 9]
            dx_t = s_const[:, 41:50]
            x_all = s_const[:, 50:51]         # [128, 1]

            # survivors of the conv/stage-B phase
            w16 = cpool.tile([128, 32, 36], F16)
            idxw = cpool.tile([128, 2304], I16)     # wrapped gather indices

            with (
                tc.tile_pool(name="conv", bufs=1) as vpool,
                tc.tile_pool(name="ps", bufs=2, space="PSUM") as pspool,
            ):
                # ---- loads ----
                s_xcl = vpool.tile([C, XCLM], F16)
                nc.sync.dma_start(out=s_xcl[:], in_=xcl[:])
                s_offk = vpool.tile([C, K * 27], F16)
                nc.sync.dma_start(out=s_offk[:], in_=offk[:])

                # ---- P1: offset conv (row-pair tiles) + P2 transposes ----
                wi_c = vpool.tile([27, 32, 128], F32)
                wiT = bpool.tile([128, 32, 27], F32)
                for b in range(NBLK):
                    q0 = (2 * b + 1) * CONVW
                    ps = pspool.tile([27, 132], F32, tag="convps")
                    for t in range(K):
                        d = int(DY[t]) * CONVW + int(DX[t])
                        nc.tensor.matmul(
                            out=ps[:],
                            lhsT=s_offk[:, t * 27:(t + 1) * 27],
                            rhs=s_xcl[:, 67 + q0 + d: 67 + q0 + 132 + d],
                            start=(t == 0), stop=(t == K - 1),
                        )
                    nc.vector.tensor_scalar(
                        out=wi_c[:, b, :].rearrange("p (r x) -> p r x", x=64),
                        in0=ps[:].rearrange("p (r x) -> p r x", x=CONVW)[:, :, 1:65],
                        scalar1=s_offb[:, 0:1], scalar2=None, op0=AL.add)
                    pt = pspool.tile([128, 27], F32, tag="wiTps")
                    nc.tensor.transpose(
                        out=pt[:], in_=wi_c[:, b, :], identity=s_eye32[:27, :27])
                    nc.vector.tensor_copy(out=wiT[:, b, :], in_=pt[:])

            # ---- P3: stage B ----
            o1 = wiT[:, :, 0:9]
            o2 = wiT[:, :, 9:18]
            mm = wiT[:, :, 18:27]
            S = [128, 32, 9]

            sigm = bpool.tile(S, F32)
            nc.scalar.activation(sigm[:], mm, mybir.ActivationFunctionType.Sigmoid)

            py = bpool.tile(S, F32, tag="py")
            nc.vector.tensor_tensor(
                out=py[:], in0=o1, in1=bcast(y_all, S),
                op=AL.add)
            nc.vector.tensor_tensor(
                out=py[:], in0=py[:], in1=bcast(dy_t.rearrange("p (o k) -> p o k", o=1), S),
                op=AL.add)
            nc.vector.tensor_scalar(out=py[:], in0=py[:], scalar1=8.0, scalar2=2.0,
                                    op0=AL.add, op1=AL.max)
            nc.vector.tensor_scalar(out=py[:], in0=py[:], scalar1=77.0, scalar2=None, op0=AL.min)
            y0p = bpool.tile(S, F32, tag="y0p")
            nc.vector.tensor_scalar(out=y0p[:], in0=py[:], scalar1=-0.5,
                                    scalar2=8388608.0, op0=AL.add, op1=AL.add)
            nc.vector.tensor_scalar(out=y0p[:], in0=y0p[:], scalar1=-8388608.0,
                                    scalar2=None, op0=AL.add)
            fy = bpool.tile(S, F32, tag="fy")
            nc.vector.tensor_tensor(out=fy[:], in0=py[:], in1=y0p[:], op=AL.subtract)
            wy0 = bpool.tile(S, F32, tag="wy0")
            nc.vector.tensor_scalar(out=wy0[:], in0=fy[:], scalar1=-1.0, scalar2=1.0,
                                    op0=AL.mult, op1=AL.add)

            px = bpool.tile(S, F32, tag="px")
            nc.vector.tensor_tensor(
                out=px[:], in0=o2,
                in1=bcast(x_all, S), op=AL.add)
            nc.vector.tensor_tensor(
                out=px[:], in0=px[:], in1=bcast(dx_t.rearrange("p (o k) -> p o k", o=1), S),
                op=AL.add)
            nc.vector.tensor_scalar(out=px[:], in0=px[:], scalar1=8.0, scalar2=2.0,
                                    op0=AL.add, op1=AL.max)
            nc.vector.tensor_scalar(out=px[:], in0=px[:], scalar1=77.0, scalar2=None, op0=AL.min)
            x0p = bpool.tile(S, F32, tag="x0p")
            nc.vector.tensor_scalar(out=x0p[:], in0=px[:], scalar1=-0.5,
                                    scalar2=8388608.0, op0=AL.add, op1=AL.add)
            nc.vector.tensor_scalar(out=x0p[:], in0=x0p[:], scalar1=-8388608.0,
                                    scalar2=None, op0=AL.add)
            fx = bpool.tile(S, F32, tag="fx")
            nc.vector.tensor_tensor(out=fx[:], in0=px[:], in1=x0p[:], op=AL.subtract)
            wx0 = bpool.tile(S, F32, tag="wx0")
            nc.vector.tensor_scalar(out=wx0[:], in0=fx[:], scalar1=-1.0, scalar2=1.0,
                                    op0=AL.mult, op1=AL.add)

            qx = bpool.tile(S, F32, tag="qx")
            nc.vector.tensor_scalar(out=qx[:], in0=x0p[:], scalar1=0.5,
                                    scalar2=-0.25, op0=AL.mult, op1=AL.add)
            nc.vector.tensor_scalar(out=qx[:], in0=qx[:], scalar1=8388608.0,
                                    scalar2=-8388608.0, op0=AL.add, op1=AL.add)
            parx = bpool.tile(S, F32, tag="parx")
            nc.vector.scalar_tensor_tensor(
                out=parx[:], in0=qx[:], scalar=-2.0, in1=x0p[:],
                op0=AL.mult, op1=AL.add)
            qy = bpool.tile(S, F32, tag="qy")
            nc.vector.tensor_scalar(out=qy[:], in0=y0p[:], scalar1=0.5,
                                    scalar2=-0.25, op0=AL.mult, op1=AL.add)
            nc.vector.tensor_scalar(out=qy[:], in0=qy[:], scalar1=8388608.0,
                                    scalar2=-8388608.0, op0=AL.add, op1=AL.add)
            pary = bpool.tile(S, F32, tag="pary")
            nc.vector.scalar_tensor_tensor(
                out=pary[:], in0=qy[:], scalar=-2.0, in1=y0p[:],
                op0=AL.mult, op1=AL.add)
            base = bpool.tile(S, F32, tag="base")
            nc.vector.scalar_tensor_tensor(
                out=base[:], in0=qy[:], scalar=40.0, in1=qx[:],
                op0=AL.mult, op1=AL.add)
            nc.vector.scalar_tensor_tensor(
                out=base[:], in0=parx[:], scalar=1600.0, in1=base[:],
                op0=AL.mult, op1=AL.add)
            nc.vector.scalar_tensor_tensor(
                out=base[:], in0=pary[:], scalar=3200.0, in1=base[:],
                op0=AL.mult, op1=AL.add)

            # on-chip wrapped-index build: idxw[q+16r, (g,pg)] = base[pg*16+q, g]
            basef = base[:].rearrange("p b k -> p (b k)")      # [128, 288]
            idxw3 = idxw[:].rearrange("p (g e) -> p g e", e=8)
            tsb = bpool.tile([128, 3, 128], F32, tag="tsb")
            with tc.tile_pool(name="psT", bufs=2, space="PSUM") as psTpool:
                for c, csz in enumerate((128, 128, 32)):
                    ptc = psTpool.tile([128, 128], F32, tag="Tc")
                    nc.tensor.transpose(
                        out=ptc[:csz, :], in_=basef[:, c * 128:c * 128 + csz],
                        identity=s_eye32[:])
                    nc.vector.tensor_copy(out=tsb[:csz, c, :], in_=ptc[:csz, :])
                for c, csz in enumerate((128, 128, 32)):
                    for pg in range(8):
                        pv = psTpool.tile([16, 128], F32, tag="V")
                        nc.tensor.transpose(
                            out=pv[:, :csz],
                            in_=tsb[:csz, c, pg * 16:(pg + 1) * 16],
                            identity=s_eye32[:csz, :csz])
                        nc.vector.tensor_copy(
                            out=idxw3[0:16, c * 128:c * 128 + csz, pg],
                            in_=pv[:, :csz])
            for lo, n in ((16, 16), (32, 32), (64, 64)):
                nc.sync.dma_start(out=idxw[lo:lo + n, :], in_=idxw[0:n, :])

            # weights W [128, 32, 9, 2, 2]  (k, yc, u)
            a0 = bpool.tile(S, F32, tag="a0")
            nc.vector.tensor_tensor(out=a0[:], in0=wy0[:], in1=sigm[:], op=AL.mult)
            a1 = bpool.tile(S, F32, tag="a1")
            nc.vector.tensor_tensor(out=a1[:], in0=fy[:], in1=sigm[:], op=AL.mult)
            w_f32 = bpool.tile([128, 32, 9, 2, 2], F32)
            nc.vector.tensor_tensor(out=w_f32[:, :, :, 0, 0], in0=a0[:], in1=wx0[:],
                                    op=AL.mult)
            nc.vector.tensor_tensor(out=w_f32[:, :, :, 0, 1], in0=a0[:], in1=fx[:],
                                    op=AL.mult)
            nc.vector.tensor_tensor(out=w_f32[:, :, :, 1, 0], in0=a1[:], in1=wx0[:],
                                    op=AL.mult)
            nc.vector.tensor_tensor(out=w_f32[:, :, :, 1, 1], in0=a1[:], in1=fx[:],
                                    op=AL.mult)
            nc.vector.tensor_copy(
                out=w16[:], in_=w_f32[:].rearrange("p b k y u -> p b (k y u)"))

            # ---- P4 ----
            with (
                tc.tile_pool(name="sgpool", bufs=2) as sgpool,
                tc.tile_pool(name="blkpool", bufs=2) as blkpool,
                tc.tile_pool(name="ps3", bufs=4, space="PSUM") as ps3pool,
            ):
                for sg in range(8):
                    dst = sgpool.tile([128, 36, 512], F16, tag="dst")
                    g0 = sg * 36
                    for cl in range(5):
                        lo = 8 * cl
                        ns = min(8, 36 - lo)
                        nc.gpsimd.dma_gather(
                            dst[:, lo:lo + ns, :], pairs[:],
                            idxw[:, (g0 + lo) * 8:(g0 + lo + ns) * 8],
                            ns * 128, ns * 128, 512)
                    cols = sgpool.tile([128, K, 512], F16, tag="cols")
                    for bi in range(4):
                        b = sg * 4 + bi
                        gw = blkpool.tile([128, 36, 128], F16, tag="gw")
                        dsrc = dst[:, 9 * bi:9 * (bi + 1), :].rearrange(
                            "p s e -> p (s e)").rearrange(
                            "p (j c) -> p j c", c=128)
                        nc.vector.tensor_tensor(
                            out=gw[:, 0:24, :], in0=dsrc[:, 0:24, :],
                            in1=bcast(w16[:, b, 0:24], [128, 24, 128]),
                            op=AL.mult)
                        nc.gpsimd.tensor_tensor(
                            out=gw[:, 24:36, :], in0=dsrc[:, 24:36, :],
                            in1=bcast(w16[:, b, 24:36], [128, 12, 128]),
                            op=AL.mult)
                        for k in range(K):
                            pc = ps3pool.tile([128, 128], F32, tag="ctps")
                            for j in range(4):
                                nc.tensor.matmul(
                                    out=pc[:], lhsT=gw[:, 4 * k + j, :],
                                    rhs=s_eye16[:], start=(j == 0), stop=(j == 3))
                            if k % 2 == 0:
                                nc.scalar.copy(
                                    out=cols[:, k, bi * 128:(bi + 1) * 128],
                                    in_=pc[:])
                            else:
                                nc.vector.tensor_copy(
                                    out=cols[:, k, bi * 128:(bi + 1) * 128],
                                    in_=pc[:])
                    for fc in range(2):
                        po = ps2pool.tile([128, 512], F32, tag="outps")
                        for k in range(K):
                            nc.tensor.matmul(
                                out=po[:],
                                lhsT=s_filt[:, (k * 2 + fc) * 128:
                                            (k * 2 + fc + 1) * 128],
                                rhs=cols[:, k, :],
                                start=(k == 0), stop=(k == K - 1))
                        osb = blkpool.tile([128, 512], F32, tag="osb")
                        if fc == 0:
                            nc.scalar.copy(out=osb[:], in_=po[:])
                        else:
                            nc.vector.tensor_copy(out=osb[:], in_=po[:])
                        nc.sync.dma_start(
                            out=out_d[fc, :, sg * 512:(sg + 1) * 512], in_=osb[:])
    nc.compile()
    return nc


def host_inputs(x, offset_kernel, offset_bias, filt_w):
    """Per-sample input maps. x [8,64,64,128] f32 etc (numpy)."""
    offk = np.ascontiguousarray(
        offset_kernel.reshape(K, C, 27).transpose(1, 0, 2).reshape(C, K * 27)
    ).astype(np.float16)
    offb = offset_bias.reshape(27, 1).astype(np.float32)
    filt_re = np.ascontiguousarray(
        filt_w.reshape(K, C, 2, 128).transpose(1, 0, 2, 3).reshape(C, K * 2 * 128)
    ).astype(np.float16)
    eye32 = np.eye(128, dtype=np.float32)
    eye16 = np.eye(128).astype(np.float16)
    consts = np.zeros((128, 51), np.float32)
    p = np.arange(128)
    yoff = p // 64
    consts[:, 0:32] = 2 * np.arange(32)[None, :] + yoff[:, None]
    consts[:, 32:41] = DY[None, :]
    consts[:, 41:50] = DX[None, :]
    consts[:, 50] = p % 64

    maps = []
    for b in range(x.shape[0]):
        xp = np.zeros((HP + 2, WP + 2, C), np.float32)
        xp[PADR:PADR + H, PADR:PADR + W] = x[b]
        quad = np.zeros((2, 2, 40, 40, 2, 2, C), np.float32)
        for pY in range(2):
            for pX in range(2):
                for uy in range(2):
                    for ux in range(2):
                        quad[pY, pX, :, :, uy, ux] = \
                            xp[pY + uy:pY + uy + 80:2, pX + ux:pX + ux + 80:2]
        prs = quad.reshape(NROWS, 4 * C).astype(np.float16)

        x1 = np.zeros((CONVW, CONVW, C), np.float32)
        x1[1:65, 1:65] = x[b]
        xcl = np.zeros((C, XCLM), np.float16)
        xcl[:, 67:67 + 4356] = x1.reshape(CONVW * CONVW, C).T.astype(np.float16)
        maps.append({
            "xcl": xcl, "pairs": prs, "offk": offk, "offb": offb,
            "filt": filt_re, "eye32": eye32, "eye16": eye16, "consts": consts,
        })
    return maps


def host_output(res_list):
    outs = []
    for r in res_list:
        o = r["out"].reshape(256, NPIX)
        outs.append(np.ascontiguousarray(o.T).reshape(H, W, F))
    return np.stack(outs)


def _get_nc():
    global _NC
    if _NC is None:
        _NC = build_nc()
    return _NC


def kernel(inputs, offset_kernel, offset_bias, filt):
    from concourse.bass_utils import run_bass_kernel_spmd
    x = np.asarray(inputs, dtype=np.float32)
    maps = host_inputs(x, np.asarray(offset_kernel, np.float32),
                       np.asarray(offset_bias, np.float32),
                       np.asarray(filt, np.float32))
    nc = _get_nc()
    res = run_bass_kernel_spmd(nc, maps, core_ids=list(range(8)))
    return host_output(res.results).astype(np.float32)



# revision 7
# speedup vs baseline: 1.6595x; 1.1103x over previous
"""DeformableConv2D (DCNv2) forward on 8 Trainium2 NeuronCores.

Data-parallel over batch: one sample per core. Per core, software-pipelined
over 8 super-groups (sg = 4 pixel blocks = 512 pixels):
  head(sg):  offset conv (PE, fp16) -> stage-B coords/weights (DVE/ACT)
             -> wrapped gather indices via PE transposes -> SWDGE desc-gen
             -> dma_gather of 2x2-patch rows
  tail(sg):  corner-weight multiply (DVE 2x-packed + gpsimd) -> accumulating
             PE transposes into banked PSUM -> im2col GEMM (PE) -> f16 store
"""
import sys
sys.path.insert(0, "/opt/trn_rl_repo")

import numpy as np
import ml_dtypes

import concourse.bass as bass
import concourse.bacc as bacc
import concourse.mybir as mybir
import concourse.tile as tile
from concourse import library_config

F32 = mybir.dt.float32
F16 = mybir.dt.float16
I16 = mybir.dt.int16
AL = mybir.AluOpType

H = W = 64
C = 128
F = 256
K = 9
PADR = 8                 # padded-coordinate margin
HP = WP = 80             # padded image
NPIX = H * W             # 4096
NBLK = 32                # pixel blocks of 128 (2 rows each)
CONVW = 66               # conv grid width (pad 1)
XCLM = 67 + 4608 + 67    # xcl with shift margins
NROWS = 2 * HP * 40      # pair-table rows = 6400
NSG = 8                  # super-groups (4 blocks each)

DY = np.repeat(np.arange(3) - 1, 3).astype(np.float32)   # per-tap dy
DX = np.tile(np.arange(3) - 1, 3).astype(np.float32)     # per-tap dx

DVE_SLOTS = 30           # corner-multiply split: slots on DVE (packed 2x)


def bcast(ap, shape):
    return ap.to_broadcast(list(shape))


_NC = None


def build_nc():
    nc = bacc.Bacc("TRN2", target_bir_lowering=False)
    xcl = nc.dram_tensor("xcl", [C, XCLM], F16, kind="ExternalInput")
    pairs = nc.dram_tensor("pairs", [NROWS, 512], F16, kind="ExternalInput")
    offk = nc.dram_tensor("offk", [C, K * 27], F16, kind="ExternalInput")
    offb = nc.dram_tensor("offb", [27, 1], F32, kind="ExternalInput")
    filt = nc.dram_tensor("filt", [C, K * 2 * 128], F16, kind="ExternalInput")
    eye32 = nc.dram_tensor("eye32", [128, 128], F32, kind="ExternalInput")
    eye16 = nc.dram_tensor("eye16", [128, 128], F16, kind="ExternalInput")
    # consts: Y_all [128,32], dy/dx rows [128,9] each, X_all [128,1]
    consts = nc.dram_tensor("consts", [128, 51], F32, kind="ExternalInput")
    out_d = nc.dram_tensor("out", [2, 128, NPIX], F16, kind="ExternalOutput")

    with tile.TileContext(nc) as tc:
        with (
            tc.tile_pool(name="const", bufs=1) as cpool,
            tc.tile_pool(name="hwork", bufs=2) as hpool,       # head tiles
            tc.tile_pool(name="idxp", bufs=2) as ipool,        # idx tiles
            tc.tile_pool(name="sgpool", bufs=2) as sgpool,     # gather dst
            tc.tile_pool(name="blkpool", bufs=2) as blkpool,   # gw / cols / osb
            tc.tile_pool(name="psh", bufs=1, space="PSUM") as pshpool,
            tc.tile_pool(name="pss", bufs=2, space="PSUM") as psspool,
            tc.tile_pool(name="psc", bufs=1, space="PSUM") as pscpool,
            tc.tile_pool(name="ps2", bufs=2, space="PSUM") as ps2pool,
        ):
            nc.gpsimd.load_library(library_config.mlp)

            s_offb = cpool.tile([27, 1], F32)
            nc.sync.dma_start(out=s_offb[:], in_=offb[:])
            s_filt = cpool.tile([C, K * 2 * 128], F16)
            nc.sync.dma_start(out=s_filt[:], in_=filt[:])
            s_eye32 = cpool.tile([128, 128], F32)
            nc.sync.dma_start(out=s_eye32[:], in_=eye32[:])
            s_eye16 = cpool.tile([128, 128], F16)
            nc.sync.dma_start(out=s_eye16[:], in_=eye16[:])
            s_const = cpool.tile([128, 51], F32)
            nc.sync.dma_start(out=s_const[:], in_=consts[:])
            y_all = s_const[:, 0:32]          # [128, 32]
            dy_t = s_const[:, 32:41]          # [128, 9]
            dx_t = s_const[:, 41:50]
            x_all = s_const[:, 50:51]         # [128, 1]
            s_xcl = cpool.tile([C, XCLM], F16)
            nc.sync.dma_start(out=s_xcl[:], in_=xcl[:])
            s_offk = cpool.tile([C, K * 27], F16)
            nc.sync.dma_start(out=s_offk[:], in_=offk[:])

            state = {}

            def head(sg):
                # ---- offset conv for 4 blocks + transpose to pixel-major ----
                wi_c = hpool.tile([27, 4, 128], F32, tag="wi_c")
                wiT = hpool.tile([128, 4, 27], F32, tag="wiT")
                for bi in range(4):
                    b = 4 * sg + bi
                    q0 = (2 * b + 1) * CONVW
                    ps = pshpool.tile([27, 132], F32, tag="convps")
                    for t in range(K):
                        d = int(DY[t]) * CONVW + int(DX[t])
                        nc.tensor.matmul(
                            out=ps[:],
                            lhsT=s_offk[:, t * 27:(t + 1) * 27],
                            rhs=s_xcl[:, 67 + q0 + d: 67 + q0 + 132 + d],
                            start=(t == 0), stop=(t == K - 1),
                        )
                    nc.vector.tensor_scalar(
                        out=wi_c[:, bi, :].rearrange("p (r x) -> p r x", x=64),
                        in0=ps[:].rearrange("p (r x) -> p r x", x=CONVW)[:, :, 1:65],
                        scalar1=s_offb[:, 0:1], scalar2=None, op0=AL.add)
                    pt = psspool.tile([128, 128], F32, tag="psmall")
                    nc.tensor.transpose(
                        out=pt[:, 0:27], in_=wi_c[:, bi, :],
                        identity=s_eye32[:27, :27])
                    nc.vector.tensor_copy(out=wiT[:, bi, :], in_=pt[:, 0:27])

                # ---- stage B: coords, bilinear weights, gather indices ----
                o1 = wiT[:, :, 0:9]
                o2 = wiT[:, :, 9:18]
                mm = wiT[:, :, 18:27]
                S = [128, 4, 9]

                sigm = hpool.tile(S, F32, tag="sigm")
                nc.scalar.activation(sigm[:], mm,
                                     mybir.ActivationFunctionType.Sigmoid)

                py = hpool.tile(S, F32, tag="py")
                nc.vector.tensor_tensor(
                    out=py[:], in0=o1,
                    in1=bcast(y_all[:, 4 * sg:4 * sg + 4], S), op=AL.add)
                nc.vector.tensor_tensor(
                    out=py[:], in0=py[:],
                    in1=bcast(dy_t.rearrange("p (o k) -> p o k", o=1), S),
                    op=AL.add)
                nc.vector.tensor_scalar(out=py[:], in0=py[:], scalar1=8.0,
                                        scalar2=2.0, op0=AL.add, op1=AL.max)
                nc.vector.tensor_scalar(out=py[:], in0=py[:], scalar1=77.0,
                                        scalar2=None, op0=AL.min)
                y0p = hpool.tile(S, F32, tag="y0p")
                nc.vector.tensor_scalar(out=y0p[:], in0=py[:], scalar1=-0.5,
                                        scalar2=8388608.0, op0=AL.add, op1=AL.add)
                nc.vector.tensor_scalar(out=y0p[:], in0=y0p[:], scalar1=-8388608.0,
                                        scalar2=None, op0=AL.add)
                fy = hpool.tile(S, F32, tag="fy")
                nc.vector.tensor_tensor(out=fy[:], in0=py[:], in1=y0p[:],
                                        op=AL.subtract)
                wy0 = hpool.tile(S, F32, tag="wy0")
                nc.vector.tensor_scalar(out=wy0[:], in0=fy[:], scalar1=-1.0,
                                        scalar2=1.0, op0=AL.mult, op1=AL.add)

                px = hpool.tile(S, F32, tag="px")
                nc.vector.tensor_tensor(
                    out=px[:], in0=o2, in1=bcast(x_all, S), op=AL.add)
                nc.vector.tensor_tensor(
                    out=px[:], in0=px[:],
                    in1=bcast(dx_t.rearrange("p (o k) -> p o k", o=1), S),
                    op=AL.add)
                nc.vector.tensor_scalar(out=px[:], in0=px[:], scalar1=8.0,
                                        scalar2=2.0, op0=AL.add, op1=AL.max)
                nc.vector.tensor_scalar(out=px[:], in0=px[:], scalar1=77.0,
                                        scalar2=None, op0=AL.min)
                x0p = hpool.tile(S, F32, tag="x0p")
                nc.vector.tensor_scalar(out=x0p[:], in0=px[:], scalar1=-0.5,
                                        scalar2=8388608.0, op0=AL.add, op1=AL.add)
                nc.vector.tensor_scalar(out=x0p[:], in0=x0p[:], scalar1=-8388608.0,
                                        scalar2=None, op0=AL.add)
                fx = hpool.tile(S, F32, tag="fx")
                nc.vector.tensor_tensor(out=fx[:], in0=px[:], in1=x0p[:],
                                        op=AL.subtract)
                wx0 = hpool.tile(S, F32, tag="wx0")
                nc.vector.tensor_scalar(out=wx0[:], in0=fx[:], scalar1=-1.0,
                                        scalar2=1.0, op0=AL.mult, op1=AL.add)

                qx = hpool.tile(S, F32, tag="qx")
                nc.vector.tensor_scalar(out=qx[:], in0=x0p[:], scalar1=0.5,
                                        scalar2=-0.25, op0=AL.mult, op1=AL.add)
                nc.vector.tensor_scalar(out=qx[:], in0=qx[:], scalar1=8388608.0,
                                        scalar2=-8388608.0, op0=AL.add, op1=AL.add)
                parx = hpool.tile(S, F32, tag="parx")
                nc.vector.scalar_tensor_tensor(
                    out=parx[:], in0=qx[:], scalar=-2.0, in1=x0p[:],
                    op0=AL.mult, op1=AL.add)
                qy = hpool.tile(S, F32, tag="qy")
                nc.vector.tensor_scalar(out=qy[:], in0=y0p[:], scalar1=0.5,
                                        scalar2=-0.25, op0=AL.mult, op1=AL.add)
                nc.vector.tensor_scalar(out=qy[:], in0=qy[:], scalar1=8388608.0,
                                        scalar2=-8388608.0, op0=AL.add, op1=AL.add)
                pary = hpool.tile(S, F32, tag="pary")
                nc.vector.scalar_tensor_tensor(
                    out=pary[:], in0=qy[:], scalar=-2.0, in1=y0p[:],
                    op0=AL.mult, op1=AL.add)
                base = hpool.tile(S, F32, tag="base")
                nc.vector.scalar_tensor_tensor(
                    out=base[:], in0=qy[:], scalar=40.0, in1=qx[:],
                    op0=AL.mult, op1=AL.add)
                nc.vector.scalar_tensor_tensor(
                    out=base[:], in0=parx[:], scalar=1600.0, in1=base[:],
                    op0=AL.mult, op1=AL.add)
                nc.vector.scalar_tensor_tensor(
                    out=base[:], in0=pary[:], scalar=3200.0, in1=base[:],
                    op0=AL.mult, op1=AL.add)

                # wrapped-index build: idxw[q+16r, (g,pg)] = base[pg*16+q, g]
                idxw = ipool.tile([128, 288], I16, tag="idxw")
                idxw3 = idxw[:].rearrange("p (g e) -> p g e", e=8)
                pTt = psspool.tile([128, 128], F32, tag="psmall")
                pT = pTt[:36, :]
                nc.tensor.transpose(
                    out=pT, in_=base[:].rearrange("p b k -> p (b k)"),
                    identity=s_eye32[:])
                tsb = hpool.tile([36, 128], F32, tag="tsb")
                nc.vector.tensor_copy(out=tsb[:], in_=pT)
                for pg in range(8):
                    pvt = psspool.tile([128, 128], F32, tag="psmall")
                    nc.tensor.transpose(
                        out=pvt[:16, 0:36], in_=tsb[:, pg * 16:(pg + 1) * 16],
                        identity=s_eye32[:36, :36])
                    nc.vector.tensor_copy(out=idxw3[0:16, :, pg],
                                          in_=pvt[:16, 0:36])
                for lo, n in ((16, 16), (32, 32), (64, 64)):
                    nc.sync.dma_start(out=idxw[lo:lo + n, :], in_=idxw[0:n, :])

                # bilinear corner weights, f16 pairs for the 2x DVE multiply
                a0 = hpool.tile(S, F32, tag="a0")
                nc.vector.tensor_tensor(out=a0[:], in0=wy0[:], in1=sigm[:],
                                        op=AL.mult)
                a1 = hpool.tile(S, F32, tag="a1")
                nc.vector.tensor_tensor(out=a1[:], in0=fy[:], in1=sigm[:],
                                        op=AL.mult)
                w_f32 = hpool.tile([128, 4, 9, 2, 2], F32, tag="wf32")
                nc.vector.tensor_tensor(out=w_f32[:, :, :, 0, 0], in0=a0[:],
                                        in1=wx0[:], op=AL.mult)
                nc.vector.tensor_tensor(out=w_f32[:, :, :, 0, 1], in0=a0[:],
                                        in1=fx[:], op=AL.mult)
                nc.vector.tensor_tensor(out=w_f32[:, :, :, 1, 0], in0=a1[:],
                                        in1=wx0[:], op=AL.mult)
                nc.vector.tensor_tensor(out=w_f32[:, :, :, 1, 1], in0=a1[:],
                                        in1=fx[:], op=AL.mult)
                w2 = ipool.tile([128, 4, 36, 2], F16, tag="w2")
                wsrc = w_f32[:].rearrange("p b k y u -> p b (k y u)")
                nc.vector.tensor_copy(
                    out=w2[:, :, :, 0:1],
                    in_=wsrc.rearrange("p b (g o) -> p b g o", o=1))
                nc.vector.tensor_copy(
                    out=w2[:, :, :, 1:2],
                    in_=wsrc.rearrange("p b (g o) -> p b g o", o=1))

                # ---- gather ----
                dst = sgpool.tile([128, 36, 512], F16, tag="dst")
                for lo, ns in ((0, 8), (8, 8), (16, 8), (24, 8), (32, 4)):
                    nc.gpsimd.dma_gather(
                        dst[:, lo:lo + ns, :], pairs[:],
                        idxw[:, lo * 8:(lo + ns) * 8],
                        ns * 128, ns * 128, 512)
                state[sg] = (dst, w2)

            def tail(sg):
                dst, w2 = state.pop(sg)
                cols = blkpool.tile([128, K, 512], F16, tag="cols")
                for bi in range(4):
                    gw = blkpool.tile([128, 36, 128], F16, tag="gw")
                    dsrc = dst[:, 9 * bi:9 * (bi + 1), :].rearrange(
                        "p s e -> p (s e)").rearrange(
                        "p (j c) -> p j c", c=128)
                    nd = DVE_SLOTS
                    nc.vector.tensor_tensor(
                        out=gw[:, 0:nd, :].rearrange("p j (b e) -> p j b e", e=2),
                        in0=dsrc[:, 0:nd, :].rearrange("p j (b e) -> p j b e", e=2),
                        in1=bcast(
                            w2[:, bi, 0:nd, :].rearrange(
                                "p (g o) e -> p g o e", o=1),
                            [128, nd, 64, 2]),
                        op=AL.mult)
                    nc.gpsimd.tensor_tensor(
                        out=gw[:, nd:36, :], in0=dsrc[:, nd:36, :],
                        in1=bcast(w2[:, bi, nd:36, 0:1], [128, 36 - nd, 128]),
                        op=AL.mult)
                    pcA = pscpool.tile([128, 512], F32, tag="pcA")
                    pcB = pscpool.tile([128, 512], F32, tag="pcB")
                    pcC = pscpool.tile([128, 128], F32, tag="pcC")
                    for k in range(K):
                        if k < 4:
                            pc = pcA[:, k * 128:(k + 1) * 128]
                        elif k < 8:
                            pc = pcB[:, (k - 4) * 128:(k - 3) * 128]
                        else:
                            pc = pcC[:]
                        for j in range(4):
                            nc.tensor.matmul(
                                out=pc, lhsT=gw[:, 4 * k + j, :],
                                rhs=s_eye16[:], start=(j == 0), stop=(j == 3))
                    nc.scalar.copy(
                        out=cols[:, 0:4, bi * 128:(bi + 1) * 128],
                        in_=pcA[:].rearrange("p (k c) -> p k c", c=128))
                    nc.scalar.copy(
                        out=cols[:, 4:8, bi * 128:(bi + 1) * 128],
                        in_=pcB[:].rearrange("p (k c) -> p k c", c=128))
                    nc.vector.tensor_copy(
                        out=cols[:, 8, bi * 128:(bi + 1) * 128], in_=pcC[:])
                for fc in range(2):
                    po = ps2pool.tile([128, 512], F32, tag="outps")
                    for k in range(K):
                        nc.tensor.matmul(
                            out=po[:],
                            lhsT=s_filt[:, (k * 2 + fc) * 128:
                                        (k * 2 + fc + 1) * 128],
                            rhs=cols[:, k, :],
                            start=(k == 0), stop=(k == K - 1))
                    osb = blkpool.tile([128, 512], F16, tag="osb")
                    nc.scalar.copy(out=osb[:], in_=po[:])
                    nc.sync.dma_start(
                        out=out_d[fc, :, sg * 512:(sg + 1) * 512], in_=osb[:])

            head(0)
            for sg in range(NSG):
                if sg + 1 < NSG:
                    head(sg + 1)
                tail(sg)
    nc.compile()
    return nc


def host_inputs(x, offset_kernel, offset_bias, filt_w):
    """Per-sample input maps. x [8,64,64,128] f32 etc (numpy)."""
    offk = np.ascontiguousarray(
        offset_kernel.reshape(K, C, 27).transpose(1, 0, 2).reshape(C, K * 27)
    ).astype(np.float16)
    offb = offset_bias.reshape(27, 1).astype(np.float32)
    filt_re = np.ascontiguousarray(
        filt_w.reshape(K, C, 2, 128).transpose(1, 0, 2, 3).reshape(C, K * 2 * 128)
    ).astype(np.float16)
    eye32 = np.eye(128, dtype=np.float32)
    eye16 = np.eye(128).astype(np.float16)
    consts = np.zeros((128, 51), np.float32)
    p = np.arange(128)
    yoff = p // 64
    consts[:, 0:32] = 2 * np.arange(32)[None, :] + yoff[:, None]
    consts[:, 32:41] = DY[None, :]
    consts[:, 41:50] = DX[None, :]
    consts[:, 50] = p % 64

    maps = []
    for b in range(x.shape[0]):
        xp = np.zeros((HP + 2, WP + 2, C), np.float32)
        xp[PADR:PADR + H, PADR:PADR + W] = x[b]
        quad = np.zeros((2, 2, 40, 40, 2, 2, C), np.float32)
        for pY in range(2):
            for pX in range(2):
                for uy in range(2):
                    for ux in range(2):
                        quad[pY, pX, :, :, uy, ux] = \
                            xp[pY + uy:pY + uy + 80:2, pX + ux:pX + ux + 80:2]
        prs = quad.reshape(NROWS, 4 * C).astype(np.float16)

        x1 = np.zeros((CONVW, CONVW, C), np.float32)
        x1[1:65, 1:65] = x[b]
        xcl = np.zeros((C, XCLM), np.float16)
        xcl[:, 67:67 + 4356] = x1.reshape(CONVW * CONVW, C).T.astype(np.float16)
        maps.append({
            "xcl": xcl, "pairs": prs, "offk": offk, "offb": offb,
            "filt": filt_re, "eye32": eye32, "eye16": eye16, "consts": consts,
        })
    return maps


def host_output(res_list):
    outs = []
    for r in res_list:
        o = r["out"].astype(np.float32).reshape(256, NPIX)
        outs.append(np.ascontiguousarray(o.T).reshape(H, W, F))
    return np.stack(outs)


def _get_nc():
    global _NC
    if _NC is None:
        _NC = build_nc()
    return _NC


def kernel(inputs, offset_kernel, offset_bias, filt):
    from concourse.bass_utils import run_bass_kernel_spmd
    x = np.asarray(inputs, dtype=np.float32)
    maps = host_inputs(x, np.asarray(offset_kernel, np.float32),
                       np.asarray(offset_bias, np.float32),
                       np.asarray(filt, np.float32))
    nc = _get_nc()
    res = run_bass_kernel_spmd(nc, maps, core_ids=list(range(8)))
    return host_output(res.results).astype(np.float32)
